# revision 31
# baseline (speedup 1.0000x reference)
"""Trainium2 Bass kernel for nn_Attention_944892805701.

Dense transformer attention layer: QKV projection + RoPE + causal GQA SDPA +
output projection. B=2, S=2048, DIM=4096, 32 Q heads / 8 KV heads, hd=128.

Sharding (8 cores): 2 (batch) x 4 (head groups). Core (b, g) computes global
Q heads [8g, 8g+8) / KV heads [2g, 2g+2) of batch b and the corresponding
partial output projection y_partial = att_heads @ Wo[:, o_slice]^T. The host
sums the 4 head-group partials per batch.

Software-pipelined schedule: the PE stream is kept dense by interleaving
"fill" matmul work into each chunk's attention item stream:

  attention(c) items  <-interleave->  fill work
  c=0: loads(1) + projections(1)
  c=1: normalize(0) + loads(2) + projections(2)
  c=2: normalize(1) + loads(3) + projections(3)
  c=3: normalize(2) + O-projection of chunks 0..2
  tail: normalize(3) + O-projection of chunk 3

Attention items run scores 2 ahead (scores_{i+2} emitted before pv_i), with
several fill matmuls between, so the ScalarE exp and DVE adds always hide
under PE work and the PE HAM clock stays at 2.4 GHz.

Softmax denominators accumulate in f32 SBUF (DVE), partition-reduced with a
ones-column f32r matmul into a per-chunk [8, 512] SBUF tile; normalization is
one reciprocal + per-head partition_broadcast + one in-place multiply of the
unnormalized bf16 attention tiles. No DRAM round-trips.
"""

import math
from contextlib import ExitStack

import numpy as np
import ml_dtypes

import concourse.bass as bass  # noqa: F401
import concourse.tile as tile
from concourse import bacc, mybir
from concourse.bass_utils import run_bass_kernel_spmd

F32 = mybir.dt.float32
F32R = mybir.dt.float32r
BF16 = mybir.dt.bfloat16
FP16 = mybir.dt.float16

N_CORES = 8
DIM = 4096
N_HEADS = 32
N_KV_HEADS = 8
HEAD_DIM = 128
SEQ = 2048

HQ = N_HEADS // 4      # 8 local q heads
HKV = N_KV_HEADS // 4  # 2 local kv heads

SC = 512
P = 128


def _r(ap):
    return ap.bitcast(F32R)


def build_program(seq=SEQ, dim=DIM, hq=HQ, hkv=HKV):
    nrep = hq // hkv
    nch = seq // SC
    ndt = dim // P
    nkt_total = seq // P
    dq = hq * HEAD_DIM
    dkv = hkv * HEAD_DIM
    scale = 1.0 / math.sqrt(HEAD_DIM)

    nc = bacc.Bacc("TRN2", target_bir_lowering=False, debug=False,
                   num_devices=N_CORES)

    xTd = nc.dram_tensor("xT", [dim, seq], BF16, kind="ExternalInput").ap()
    wqT = nc.dram_tensor("wqT", [dim, dq], BF16, kind="ExternalInput").ap()
    wkT = nc.dram_tensor("wkT", [dim, dkv], BF16, kind="ExternalInput").ap()
    wvT = nc.dram_tensor("wvT", [dim, dkv], BF16, kind="ExternalInput").ap()
    wot = nc.dram_tensor("wot", [dim // P, dq, P], BF16,
                         kind="ExternalInput").ap()
    cosT = nc.dram_tensor("cosT", [HEAD_DIM, seq], F32,
                          kind="ExternalInput").ap()
    sinT = nc.dram_tensor("sinT", [HEAD_DIM, seq], F32,
                          kind="ExternalInput").ap()
    tri = nc.dram_tensor("tri", [P, P], FP16, kind="ExternalInput").ap()
    iden = nc.dram_tensor("iden", [P, P], BF16, kind="ExternalInput").ap()
    ones_col = nc.dram_tensor("ones_col", [P, 1], F32R,
                              kind="ExternalInput").ap()
    ones16 = nc.dram_tensor("ones16", [P, 1], FP16,
                            kind="ExternalInput").ap()
    outT = nc.dram_tensor("outT", [dim, seq], BF16,
                          kind="ExternalOutput").ap()

    with ExitStack() as ctx:
        tc = ctx.enter_context(tile.TileContext(nc))
        # persistent per-chunk tensors
        qp = ctx.enter_context(tc.tile_pool(name="qp", bufs=nch * hq))
        kp = ctx.enter_context(tc.tile_pool(name="kp", bufs=nch * hkv))
        vp = ctx.enter_context(tc.tile_pool(name="vp", bufs=hkv * nkt_total))
        aop = ctx.enter_context(tc.tile_pool(name="aop", bufs=nch * hq))
        dnp = ctx.enter_context(tc.tile_pool(name="dnp", bufs=2 * nch))
        rowp = ctx.enter_context(tc.tile_pool(name="rowp", bufs=5))
        # rings
        xp = ctx.enter_context(tc.tile_pool(name="xp", bufs=33))   # bf16 512
        etp = ctx.enter_context(tc.tile_pool(name="etp", bufs=4))  # bf16 512
        csp = ctx.enter_context(tc.tile_pool(name="csp", bufs=4))  # f32 cos/sin
        accp = ctx.enter_context(tc.tile_pool(name="accp", bufs=2))
        ws = ctx.enter_context(tc.tile_pool(name="ws", bufs=4))    # f32 512
        rbp = ctx.enter_context(tc.tile_pool(name="rbp", bufs=2))  # f32 bcast
        vsb = ctx.enter_context(tc.tile_pool(name="vsb", bufs=2))  # bf16 512
        wqp = ctx.enter_context(tc.tile_pool(name="wqp", bufs=5))
        wkvp = ctx.enter_context(tc.tile_pool(name="wkvp", bufs=8))
        wop = ctx.enter_context(tc.tile_pool(name="wop", bufs=3))
        cns = ctx.enter_context(tc.tile_pool(name="cns", bufs=1))
        ps_p = ctx.enter_context(tc.tile_pool(name="ps_p", bufs=2,
                                              space="PSUM"))
        ps_s = ctx.enter_context(tc.tile_pool(name="ps_s", bufs=3,
                                              space="PSUM"))
        ps_o = ctx.enter_context(tc.tile_pool(name="ps_o", bufs=2,
                                              space="PSUM"))
        ps_t = ctx.enter_context(tc.tile_pool(name="ps_t", bufs=1,
                                              space="PSUM"))

        tri_sb = cns.tile([P, P], FP16, tag="tri")
        nc.scalar.dma_start(tri_sb[:], tri[:])
        iden_sb = cns.tile([P, P], BF16, tag="iden")
        nc.scalar.dma_start(iden_sb[:], iden[:])
        ones_sb = cns.tile([P, 1], F32R, tag="ones")
        nc.scalar.dma_start(ones_sb[:], ones_col[:])
        ones16_sb = cns.tile([P, 1], FP16, tag="ones16")
        nc.scalar.dma_start(ones16_sb[:], ones16[:])

        qT = [[qp.tile([P, SC], BF16, tag="q", name=f"qT{c}_{h}")
               for h in range(hq)] for c in range(nch)]
        kT = [[kp.tile([P, SC], BF16, tag="k", name=f"kT{c}_{g}")
               for g in range(hkv)] for c in range(nch)]
        v_nat = [[vp.tile([P, HEAD_DIM], BF16, tag="v", name=f"v{g}_{t}")
                  for t in range(nkt_total)] for g in range(hkv)]
        ao = [[aop.tile([P, SC], BF16, tag="ao", name=f"ao{h}_{cc}")
               for cc in range(nch)] for h in range(hq)]
        # per-chunk denominator tiles on partitions 0..hq-1; per-head rows
        # land there via SBUF->SBUF DMA (engines need aligned partition
        # bases, DMAs don't)
        dn_c = [dnp.tile([hq, SC], F32, tag="dn", name=f"dn{c}")
                for c in range(nch)]
        rc_c = [dnp.tile([hq, SC], F32, tag="dn", name=f"rc{c}")
                for c in range(nch)]

        warm = cns.tile([P, SC], BF16, tag="warm")
        nc.vector.memset(warm[:], 0)
        for wi in range(16):
            pw = ps_s.tile([P, SC], F32, tag="s", name=f"warm{wi}")
            nc.tensor.matmul(pw[:], warm[:, 0:P], warm[:],
                             start=True, stop=True)

        def rope_drain(dst, psum, cos_c, sin_c):
            """dst(bf16) = psum*cos + rotate_half(psum)*sin."""
            h = HEAD_DIM // 2
            tmp = ws.tile([P, SC], F32, tag="ws")
            nc.vector.tensor_mul(dst, psum, cos_c[:])
            nc.vector.tensor_mul(tmp[0:h, :], psum[h:P, :], sin_c[0:h, :])
            nc.vector.tensor_mul(tmp[h:P, :], psum[0:h, :], sin_c[h:P, :])
            nc.vector.tensor_sub(dst[0:h, :], dst[0:h, :], tmp[0:h, :])
            nc.vector.tensor_add(dst[h:P, :], dst[h:P, :], tmp[h:P, :])

        def proj_quanta(c):
            """Fill-work quanta for loads + projections of chunk c.

            Each quantum is a closure emitting a small batch of DMAs or
            ~8 matmuls. Load quanta are interleaved ~3 ahead of the
            compute quanta that consume them.
            """
            s0 = c * SC
            st = {"xT": [None] * ndt, "wq": {},
                  "wk": [None] * (ndt // 4), "wv": [None] * (ndt // 4)}
            # dense chunk 0 alternates psum rings (attention rings are idle
            # then) so a group's rope drain hides under the next group's MMs
            pools = [ps_p, ps_s] if c == 0 else [ps_p]
            pidx = [0]

            def next_pool():
                p = pools[pidx[0] % len(pools)]
                pidx[0] += 1
                return p, ("p" if p is ps_p else "s")

            def q_cos():
                st["cos"] = csp.tile([P, SC], F32, tag="cs", name=f"cos{c}")
                nc.sync.dma_start(st["cos"][:], cosT[:, s0:s0 + SC])
                st["sin"] = csp.tile([P, SC], F32, tag="cs", name=f"sin{c}")
                nc.sync.dma_start(st["sin"][:], sinT[:, s0:s0 + SC])

            def q_load_x(dt4):
                def f():
                    for j in range(4):
                        dt = dt4 * 4 + j
                        t = xp.tile([P, SC], BF16, tag="xt",
                                    name=f"xT{c}_{dt}")
                        nc.sync.dma_start(t[:],
                                          xTd[dt * P:(dt + 1) * P,
                                              s0:s0 + SC])
                        st["xT"][dt] = t
                return f

            def q_load_wq(hb, dt4):
                def f():
                    wq = wqp.tile([P, 4, 2 * HEAD_DIM], BF16, tag="wq",
                                  name=f"wq{c}_{hb}_{dt4}")
                    eng = nc.scalar if (c == 0 and hb == 0 and dt4 == 0) \
                        else nc.sync
                    eng.dma_start(
                        wq[:], wqT[dt4 * 4 * P:(dt4 + 1) * 4 * P,
                                   hb * HEAD_DIM:(hb + 2) * HEAD_DIM
                                   ].rearrange("(d p) f -> p d f", p=P))
                    st["wq"][(hb, dt4)] = wq
                return f

            def q_load_wkv(which, dt4):
                def f():
                    w = wkvp.tile([P, 4, dkv], BF16, tag="wkv",
                                  name=f"w{which}{c}_{dt4}")
                    src = wkT if which == "k" else wvT
                    nc.sync.dma_start(
                        w[:], src[dt4 * 4 * P:(dt4 + 1) * 4 * P, :
                                  ].rearrange("(d p) f -> p d f", p=P))
                    st["w" + which][dt4] = w
                return f

            def q_mm_q(hb, dt4):
                def f():
                    if dt4 == 0:
                        pool, ptag = next_pool()
                        st[("pq", hb)] = [
                            pool.tile([P, SC], F32, tag=ptag,
                                      name=f"pq{c}_{hb}_{i}")
                            for i in range(2)]
                    pqs = st[("pq", hb)]
                    wq = st["wq"][(hb, dt4)]
                    for j in range(4):
                        dt = dt4 * 4 + j
                        for i in range(2):
                            nc.tensor.matmul(
                                pqs[i][:],
                                wq[:, j, i * HEAD_DIM:(i + 1) * HEAD_DIM],
                                st["xT"][dt][:],
                                start=(dt == 0), stop=(dt == ndt - 1))
                    if dt4 == ndt // 4 - 1:
                        for i in range(2):
                            rope_drain(qT[c][hb + i][:], pqs[i][:],
                                       st["cos"], st["sin"])
                        del st[("pq", hb)]
                return f

            def q_mm_kv(which, dt4):
                def f():
                    key = ("pk",) if which == "k" else ("pv",)
                    if dt4 == 0:
                        pool, ptag = next_pool()
                        st[key] = [
                            pool.tile([P, SC], F32, tag=ptag,
                                      name=f"p{which}{c}_{g}")
                            for g in range(hkv)]
                    ps = st[key]
                    w = st["w" + which][dt4]
                    for j in range(4):
                        dt = dt4 * 4 + j
                        for g in range(hkv):
                            nc.tensor.matmul(
                                ps[g][:],
                                w[:, j, g * HEAD_DIM:(g + 1) * HEAD_DIM],
                                st["xT"][dt][:],
                                start=(dt == 0), stop=(dt == ndt - 1))
                    if dt4 == ndt // 4 - 1:
                        if which == "k":
                            for g in range(hkv):
                                rope_drain(kT[c][g][:], ps[g][:],
                                           st["cos"], st["sin"])
                        else:
                            for g in range(hkv):
                                vt_sb = vsb.tile([P, SC], BF16, tag="vs")
                                nc.vector.tensor_copy(vt_sb[:], ps[g][:])
                                for t in range(SC // P):
                                    pt = ps_t.tile([P, P], BF16, tag="t")
                                    nc.tensor.transpose(
                                        pt[:], vt_sb[:, t * P:(t + 1) * P],
                                        iden_sb[:])
                                    nc.vector.tensor_copy(
                                        v_nat[g][c * (SC // P) + t][:], pt[:])
                        del st[key]
                return f

            # Build the quantum list. Loads appear in consumption order;
            # need[ci] = index of the last load that comps[ci] depends on.
            # Chunk 0 runs Q,K,V (attention starts with Q scores right
            # after); fill chunks run V,K,Q so the V-transpose chain and K
            # drains land early in the phase, never at the flush boundary.
            group_order = ["q", "k", "v"] if c == 0 else ["v", "k", "q"]
            loads = [q_load_x(0)]
            comps = []
            need = []

            def add(comp, *ls):
                loads.extend(ls)
                comps.append(comp)
                need.append(len(loads) - 1)

            for gi, grp in enumerate(group_order):
                if grp == "q":
                    for hb in range(0, hq, 2):
                        for dt4 in range(ndt // 4):
                            ls = [q_load_wq(hb, dt4)]
                            if gi == 0 and hb == 0 and dt4 == 0:
                                ls.append(q_cos)
                            if gi == 0 and hb == 0 and dt4 + 1 < ndt // 4:
                                ls.append(q_load_x(dt4 + 1))
                            add(q_mm_q(hb, dt4), *ls)
                else:
                    for dt4 in range(ndt // 4):
                        ls = [q_load_wkv(grp, dt4)]
                        if gi == 0 and dt4 == 0:
                            ls.append(q_cos)
                        if gi == 0 and dt4 + 1 < ndt // 4:
                            ls.append(q_load_x(dt4 + 1))
                        add(q_mm_kv(grp, dt4), *ls)
            quanta = []
            li = 0
            for ci, comp in enumerate(comps):
                # emit every load this comp needs, plus prefetch the loads
                # the NEXT comp needs (so DMA has a head start); no
                # prefetch before the very first comp (startup latency)
                tgt = need[ci] if ci == 0 else need[min(ci + 1, len(comps) - 1)]
                while li <= tgt:
                    quanta.append(loads[li])
                    li += 1
                quanta.append(comp)
            quanta.extend(loads[li:])
            return quanta

        def norm_quanta(cc, heads=range(hq), batched_recip=True):
            """Normalize ao[h][cc] in place: one reciprocal of the [8,512]
            denominator tile + per-head broadcast and multiply."""
            q = []
            heads = list(heads)
            h1 = heads[-1] + 1

            def recip():
                nc.vector.reciprocal(rc_c[cc][0:h1, :], dn_c[cc][0:h1, :])
            q.append(recip)
            for h in heads:
                def bm(h=h):
                    rrow = rowp.tile([1, SC], F32, tag="rrow", bufs=2)
                    nc.sync.dma_start(rrow[0:1, :], rc_c[cc][h:h + 1, :])
                    rb = rbp.tile([P, SC], F32, tag="rb")
                    nc.gpsimd.partition_broadcast(rb[:], rrow[0:1, :])
                    nc.vector.tensor_mul(ao[h][cc][:], ao[h][cc][:], rb[:])
                q.append(bm)
            return q

        def oproj_quanta(ccs):
            """O-projection quanta for the given chunks, m-tile major.

            wo loads run 2 m-tiles ahead of their matmuls, alternating the
            sync/scalar HWDGE queues; bf16 output halves store traffic."""
            st = {}

            def load(m):
                def f():
                    wo = wop.tile([P, hq, P], BF16, tag="wo",
                                  name=f"wo{m}_{ccs[0]}")
                    eng = nc.sync if m % 2 == 0 else nc.scalar
                    eng.dma_start(
                        wo[:], wot[m].rearrange("(o p) f -> p o f", p=P))
                    st[m] = wo
                return f

            def mm(m, cc):
                def f():
                    wo = st[m]
                    py = ps_p.tile([P, SC], F32, tag="p")
                    for o in range(hq):
                        nc.tensor.matmul(
                            py[:], wo[:, o, :], ao[o][cc][:],
                            start=(o == 0), stop=(o == hq - 1))
                    yo = vsb.tile([P, SC], BF16, tag="yo", bufs=4)
                    nc.vector.tensor_copy(yo[:], py[:])
                    eng = nc.scalar if m % 2 == 0 else nc.sync
                    eng.dma_start(
                        outT[m * P:(m + 1) * P, cc * SC:(cc + 1) * SC],
                        yo[:])
                return f

            nm = dim // P
            q = [load(0), load(1)]
            for m in range(nm):
                for cc in ccs:
                    q.append(mm(m, cc))
                if m + 2 < nm:
                    q.append(load(m + 2))
            return q

        def emit_attention(c, fill, reserve=0):
            s0 = c * SC
            nkt = (c + 1) * (SC // P)
            items = [(h, kt) for h in range(hq) for kt in range(nkt)]
            n_items = len(items)
            nf = len(fill)
            npaced = max(0, nf - reserve)
            fstate = [0]

            def drain_fill(i):
                tgt = npaced * (i + 1) // n_items
                while fstate[0] < tgt:
                    fill[fstate[0]]()
                    fstate[0] += 1

            pipe = {}

            def sc_ahead(i):
                h2, kt2 = items[i]
                g2 = h2 // nrep
                jlo = max(0, kt2 * P - s0)
                kc, ko = divmod(kt2, SC // P)
                pscr = ps_s.tile([P, SC], F32, tag="s",
                                 name=f"pscr{c}_{h2}_{kt2}")
                nc.tensor.matmul(
                    pscr[:, jlo:SC],
                    kT[kc][g2][:, ko * P:(ko + 1) * P],
                    qT[c][h2][:, jlo:SC],
                    start=True, stop=True)
                pipe[i] = pscr

            def head_end(h, acc, po):
                """Denominator reduce + drain for a finished head."""
                pd = ps_s.tile([P, SC], F32, tag="s", name=f"pd{c}_{h}")
                nc.tensor.matmul(pd[0:1, :], ones16_sb[:], acc[:],
                                 start=True, stop=True)
                dps = rowp.tile([1, SC], F32, tag="dps", bufs=3)
                nc.scalar.copy(dps[0:1, :], pd[0:1, :])
                nc.sync.dma_start(dn_c[c][h:h + 1, :], dps[0:1, :])
                nc.vector.tensor_copy(ao[h][c][:], po[:])
                return dps

            sc_ahead(0)
            if len(items) > 1:
                sc_ahead(1)
            accs = {}
            pos = {}
            last_dps = [None]
            for i, (h, kt) in enumerate(items):
                g = h // nrep
                if kt == 0:
                    accs[h] = accp.tile([P, SC], FP16, tag="acc",
                                        name=f"acc{c}_{h}")
                    pos[h] = ps_o.tile([P, SC], F32, tag="o",
                                       name=f"po{c}_{h}")
                acc, po = accs[h], pos[h]
                rr = kt * P - s0
                jlo = max(0, rr)
                if i + 2 < len(items):
                    sc_ahead(i + 2)
                pscr = pipe.pop(i)
                et = etp.tile([P, SC], FP16, tag="et",
                              name=f"et{c}_{h}_{kt}")
                nc.scalar.activation(
                    et[:, jlo:SC], pscr[:, jlo:SC],
                    mybir.ActivationFunctionType.Exp, scale=scale)
                if rr >= 0:
                    nc.vector.tensor_mul(et[:, jlo:jlo + P],
                                         et[:, jlo:jlo + P], tri_sb[:])
                if kt == 0:
                    nc.vector.tensor_copy(acc[:], et[:])
                else:
                    nc.vector.tensor_add(acc[:, jlo:SC],
                                         acc[:, jlo:SC], et[:, jlo:SC])
                nc.tensor.matmul(
                    po[:, jlo:SC],
                    v_nat[g][kt][:],
                    et[:, jlo:SC],
                    start=(kt == 0), stop=(kt == nkt - 1))
                if kt == nkt - 1 and h < hq - 1:
                    # fill first: gives ACT exp + DVE add time to finish
                    # before the PE's ones-matmul consumes acc
                    drain_fill(i)
                    head_end(h, acc, po)
                    del accs[h], pos[h]
                    if c == nch - 1 and h == hq - 2:
                        # heads 0..hq-2 done: normalize them while the
                        # last head's items run
                        for f in norm_quanta(c, range(hq - 1)):
                            f()
                elif kt == nkt - 1:
                    # final head: flush the reserved fill first so the
                    # denominator chain overlaps real PE work
                    while fstate[0] < nf:
                        fill[fstate[0]]()
                        fstate[0] += 1
                    if c == nch - 1:
                        last_dps[0] = head_end(h, acc, po)
                    else:
                        # defer into the next chunk's fill: the pd matmul
                        # then sits behind the next chunk's first scores
                        def deferred(h=h, acc=acc, po=po):
                            head_end(h, acc, po)
                        last_dps[0] = deferred
                    del accs[h], pos[h]
                else:
                    drain_fill(i)
            while fstate[0] < nf:
                fill[fstate[0]]()
                fstate[0] += 1
            return last_dps[0]

        # ---- schedule ----
        for f in proj_quanta(0):
            f()
        dps_last = None
        for c in range(nch):
            if c < nch - 1:
                fill = list(proj_quanta(c + 1))
                if c >= 1:
                    fill = norm_quanta(c - 1) + fill
            else:
                fill = norm_quanta(c - 1) + oproj_quanta([0, 1, 2])
            if dps_last is not None:
                fill = [dps_last] + fill
            dps_last = emit_attention(c, fill, reserve=max(8, len(fill) // 8))
        # tail: last head of last chunk (reciprocal straight off the
        # partition-0 scratch row, no SBUF round-trip) + O-proj of chunk 3
        rlast = rowp.tile([1, SC], F32, tag="rrow", bufs=2)
        nc.vector.reciprocal(rlast[0:1, :], dps_last[0:1, :])
        rb = rbp.tile([P, SC], F32, tag="rb")
        nc.gpsimd.partition_broadcast(rb[:], rlast[0:1, :])
        nc.vector.tensor_mul(ao[hq - 1][nch - 1][:],
                             ao[hq - 1][nch - 1][:], rb[:])
        for f in oproj_quanta([nch - 1]):
            f()

    nc.compile()
    return nc


def make_core_inputs(data, Wq, Wk, Wv, Wo, cos, sin):
    """Build in_maps for the 8 cores. Core id = 4*b + g."""
    bf = ml_dtypes.bfloat16

    def cbf(a):
        return np.ascontiguousarray(np.asarray(a).astype(bf))

    c = np.ascontiguousarray
    dq = HQ * HEAD_DIM
    dkv = HKV * HEAD_DIM
    dim = Wq.shape[1]
    tri_m = np.triu(np.ones((P, P), dtype=np.float16))
    iden = np.eye(P, dtype=bf)
    ones_col = np.ones((P, 1), dtype=np.float32)
    ones16 = np.ones((P, 1), dtype=np.float16)
    cosT = c(cos.T.astype(np.float32))
    sinT = c(sin.T.astype(np.float32))
    xt_by_batch = [cbf(data[b].T) for b in range(data.shape[0])]
    in_maps = []
    for core in range(N_CORES):
        b, g = divmod(core, 4)
        qs = slice(g * dq, (g + 1) * dq)
        ks = slice(g * dkv, (g + 1) * dkv)
        woT = Wo[:, qs].T                        # [dq, dim]
        wot = cbf(woT.reshape(dq, dim // P, P).transpose(1, 0, 2))
        in_maps.append({
            "xT": xt_by_batch[b],
            "wqT": cbf(Wq[qs, :].T),
            "wkT": cbf(Wk[ks, :].T),
            "wvT": cbf(Wv[ks, :].T),
            "wot": wot,
            "cosT": cosT,
            "sinT": sinT,
            "tri": tri_m,
            "iden": iden,
            "ones_col": ones_col,
            "ones16": ones16,
        })
    return in_maps


_COMPILED = {}


def _get_program():
    key = (SEQ, DIM, HQ, HKV)
    if key not in _COMPILED:
        _COMPILED[key] = build_program()
    return _COMPILED[key]


def run(inputs, trace=False, tmpdir=None, trace_cores=None):
    nc = _get_program()
    in_maps = make_core_inputs(
        inputs["data"], inputs["Wq"], inputs["Wk"], inputs["Wv"],
        inputs["Wo"], inputs["cos"], inputs["sin"])
    kw = {}
    if trace:
        kw = dict(trace=True, tmpdir=tmpdir, trace_cores=trace_cores)
    res = run_bass_kernel_spmd(nc, in_maps, list(range(N_CORES)), **kw)
    B = inputs["data"].shape[0]
    out = np.zeros((B, SEQ, DIM), dtype=np.float32)
    for core in range(N_CORES):
        b = core // 4
        out[b] += np.asarray(res.results[core]["outT"]).astype(np.float32).T
    return out, res


def kernel(data, Wq, Wk, Wv, Wo, cos, sin, mask):
    assert np.asarray(mask).size == 1, "only causal (numel==1) mask supported"
    inputs = {
        "data": np.asarray(data, dtype=np.float32),
        "Wq": np.asarray(Wq, dtype=np.float32),
        "Wk": np.asarray(Wk, dtype=np.float32),
        "Wv": np.asarray(Wv, dtype=np.float32),
        "Wo": np.asarray(Wo, dtype=np.float32),
        "cos": np.asarray(cos, dtype=np.float32),
        "sin": np.asarray(sin, dtype=np.float32),
    }
    out, _ = run(inputs)
    return out


# revision 32
# speedup vs baseline: 1.0031x; 1.0031x over previous
"""Trainium2 Bass kernel for nn_Attention_944892805701.

Dense transformer attention layer: QKV projection + RoPE + causal GQA SDPA +
output projection. B=2, S=2048, DIM=4096, 32 Q heads / 8 KV heads, hd=128.

Sharding (8 cores): 2 (batch) x 4 (head groups). Core (b, g) computes global
Q heads [8g, 8g+8) / KV heads [2g, 2g+2) of batch b and the corresponding
partial output projection y_partial = att_heads @ Wo[:, o_slice]^T. The host
sums the 4 head-group partials per batch.

Software-pipelined schedule: the PE stream is kept dense by interleaving
"fill" matmul work into each chunk's attention item stream:

  attention(c) items  <-interleave->  fill work
  c=0: loads(1) + projections(1)
  c=1: normalize(0) + loads(2) + projections(2)
  c=2: normalize(1) + loads(3) + projections(3)
  c=3: normalize(2) + O-projection of chunks 0..2
  tail: normalize(3) + O-projection of chunk 3

Attention items run scores 2 ahead (scores_{i+2} emitted before pv_i), with
several fill matmuls between, so the ScalarE exp and DVE adds always hide
under PE work and the PE HAM clock stays at 2.4 GHz.

Softmax denominators accumulate in f32 SBUF (DVE), partition-reduced with a
ones-column f32r matmul into a per-chunk [8, 512] SBUF tile; normalization is
one reciprocal + per-head partition_broadcast + one in-place multiply of the
unnormalized bf16 attention tiles. No DRAM round-trips.
"""

import math
from contextlib import ExitStack

import numpy as np
import ml_dtypes

import concourse.bass as bass  # noqa: F401
import concourse.tile as tile
from concourse import bacc, mybir
from concourse.bass_utils import run_bass_kernel_spmd

F32 = mybir.dt.float32
F32R = mybir.dt.float32r
BF16 = mybir.dt.bfloat16
FP16 = mybir.dt.float16

N_CORES = 8
DIM = 4096
N_HEADS = 32
N_KV_HEADS = 8
HEAD_DIM = 128
SEQ = 2048

HQ = N_HEADS // 4      # 8 local q heads
HKV = N_KV_HEADS // 4  # 2 local kv heads

SC = 512
P = 128


def _r(ap):
    return ap.bitcast(F32R)


def build_program(seq=SEQ, dim=DIM, hq=HQ, hkv=HKV):
    nrep = hq // hkv
    nch = seq // SC
    ndt = dim // P
    nkt_total = seq // P
    dq = hq * HEAD_DIM
    dkv = hkv * HEAD_DIM
    scale = 1.0 / math.sqrt(HEAD_DIM)

    nc = bacc.Bacc("TRN2", target_bir_lowering=False, debug=False,
                   num_devices=N_CORES)

    xTd = nc.dram_tensor("xT", [dim, seq], BF16, kind="ExternalInput").ap()
    wqT = nc.dram_tensor("wqT", [dim, dq], BF16, kind="ExternalInput").ap()
    wkT = nc.dram_tensor("wkT", [dim, dkv], BF16, kind="ExternalInput").ap()
    wvT = nc.dram_tensor("wvT", [dim, dkv], BF16, kind="ExternalInput").ap()
    wot = nc.dram_tensor("wot", [dim // P, dq, P], BF16,
                         kind="ExternalInput").ap()
    cosT = nc.dram_tensor("cosT", [HEAD_DIM, seq], F32,
                          kind="ExternalInput").ap()
    sinT = nc.dram_tensor("sinT", [HEAD_DIM, seq], F32,
                          kind="ExternalInput").ap()
    tri = nc.dram_tensor("tri", [P, P], FP16, kind="ExternalInput").ap()
    iden = nc.dram_tensor("iden", [P, P], BF16, kind="ExternalInput").ap()
    ones_col = nc.dram_tensor("ones_col", [P, 1], F32R,
                              kind="ExternalInput").ap()
    ones16 = nc.dram_tensor("ones16", [P, 1], FP16,
                            kind="ExternalInput").ap()
    outT = nc.dram_tensor("outT", [dim, seq], BF16,
                          kind="ExternalOutput").ap()

    with ExitStack() as ctx:
        tc = ctx.enter_context(tile.TileContext(nc))
        # persistent per-chunk tensors
        qp = ctx.enter_context(tc.tile_pool(name="qp", bufs=nch * hq))
        kp = ctx.enter_context(tc.tile_pool(name="kp", bufs=nch * hkv))
        vp = ctx.enter_context(tc.tile_pool(name="vp", bufs=hkv * nkt_total))
        aop = ctx.enter_context(tc.tile_pool(name="aop", bufs=nch * hq))
        dnp = ctx.enter_context(tc.tile_pool(name="dnp", bufs=2 * nch))
        rowp = ctx.enter_context(tc.tile_pool(name="rowp", bufs=5))
        # rings
        xp = ctx.enter_context(tc.tile_pool(name="xp", bufs=33))   # bf16 512
        etp = ctx.enter_context(tc.tile_pool(name="etp", bufs=4))  # bf16 512
        csp = ctx.enter_context(tc.tile_pool(name="csp", bufs=4))  # f32 cos/sin
        accp = ctx.enter_context(tc.tile_pool(name="accp", bufs=2))
        ws = ctx.enter_context(tc.tile_pool(name="ws", bufs=4))    # f32 512
        rbp = ctx.enter_context(tc.tile_pool(name="rbp", bufs=2))  # f32 bcast
        vsb = ctx.enter_context(tc.tile_pool(name="vsb", bufs=2))  # bf16 512
        wqp = ctx.enter_context(tc.tile_pool(name="wqp", bufs=5))
        wkvp = ctx.enter_context(tc.tile_pool(name="wkvp", bufs=8))
        wop = ctx.enter_context(tc.tile_pool(name="wop", bufs=3))
        cns = ctx.enter_context(tc.tile_pool(name="cns", bufs=1))
        ps_p = ctx.enter_context(tc.tile_pool(name="ps_p", bufs=2,
                                              space="PSUM"))
        ps_s = ctx.enter_context(tc.tile_pool(name="ps_s", bufs=3,
                                              space="PSUM"))
        ps_o = ctx.enter_context(tc.tile_pool(name="ps_o", bufs=2,
                                              space="PSUM"))
        ps_t = ctx.enter_context(tc.tile_pool(name="ps_t", bufs=1,
                                              space="PSUM"))

        tri_sb = cns.tile([P, P], FP16, tag="tri")
        nc.scalar.dma_start(tri_sb[:], tri[:])
        iden_sb = cns.tile([P, P], BF16, tag="iden")
        nc.scalar.dma_start(iden_sb[:], iden[:])
        ones_sb = cns.tile([P, 1], F32R, tag="ones")
        nc.scalar.dma_start(ones_sb[:], ones_col[:])
        ones16_sb = cns.tile([P, 1], FP16, tag="ones16")
        nc.scalar.dma_start(ones16_sb[:], ones16[:])

        qT = [[qp.tile([P, SC], BF16, tag="q", name=f"qT{c}_{h}")
               for h in range(hq)] for c in range(nch)]
        kT = [[kp.tile([P, SC], BF16, tag="k", name=f"kT{c}_{g}")
               for g in range(hkv)] for c in range(nch)]
        v_nat = [[vp.tile([P, HEAD_DIM], BF16, tag="v", name=f"v{g}_{t}")
                  for t in range(nkt_total)] for g in range(hkv)]
        ao = [[aop.tile([P, SC], BF16, tag="ao", name=f"ao{h}_{cc}")
               for cc in range(nch)] for h in range(hq)]
        # per-chunk denominator tiles on partitions 0..hq-1; per-head rows
        # land there via SBUF->SBUF DMA (engines need aligned partition
        # bases, DMAs don't)
        dn_c = [dnp.tile([hq, SC], F32, tag="dn", name=f"dn{c}")
                for c in range(nch)]
        rc_c = [dnp.tile([hq, SC], F32, tag="dn", name=f"rc{c}")
                for c in range(nch)]

        warm = cns.tile([P, SC], BF16, tag="warm")
        nc.vector.memset(warm[:], 0)
        for wi in range(16):
            pw = ps_s.tile([P, SC], F32, tag="s", name=f"warm{wi}")
            nc.tensor.matmul(pw[:], warm[:, 0:P], warm[:],
                             start=True, stop=True)

        def rope_drain(dst, psum, cos_c, sin_c):
            """dst(bf16) = psum*cos + rotate_half(psum)*sin."""
            h = HEAD_DIM // 2
            tmp = ws.tile([P, SC], F32, tag="ws")
            nc.vector.tensor_mul(dst, psum, cos_c[:])
            nc.vector.tensor_mul(tmp[0:h, :], psum[h:P, :], sin_c[0:h, :])
            nc.vector.tensor_mul(tmp[h:P, :], psum[0:h, :], sin_c[h:P, :])
            nc.vector.tensor_sub(dst[0:h, :], dst[0:h, :], tmp[0:h, :])
            nc.vector.tensor_add(dst[h:P, :], dst[h:P, :], tmp[h:P, :])

        def proj_quanta(c):
            """Fill-work quanta for loads + projections of chunk c.

            Each quantum is a closure emitting a small batch of DMAs or
            ~8 matmuls. Load quanta are interleaved ~3 ahead of the
            compute quanta that consume them.
            """
            s0 = c * SC
            st = {"xT": [None] * ndt, "wq": {},
                  "wk": [None] * (ndt // 4), "wv": [None] * (ndt // 4)}
            # dense chunk 0 alternates psum rings (attention rings are idle
            # then) so a group's rope drain hides under the next group's MMs
            pools = [ps_p, ps_s] if c == 0 else [ps_p]
            pidx = [0]

            def next_pool():
                p = pools[pidx[0] % len(pools)]
                pidx[0] += 1
                return p, ("p" if p is ps_p else "s")

            def q_cos():
                st["cos"] = csp.tile([P, SC], F32, tag="cs", name=f"cos{c}")
                nc.sync.dma_start(st["cos"][:], cosT[:, s0:s0 + SC])
                st["sin"] = csp.tile([P, SC], F32, tag="cs", name=f"sin{c}")
                nc.sync.dma_start(st["sin"][:], sinT[:, s0:s0 + SC])

            def q_load_x(dt4):
                def f():
                    for j in range(4):
                        dt = dt4 * 4 + j
                        t = xp.tile([P, SC], BF16, tag="xt",
                                    name=f"xT{c}_{dt}")
                        nc.sync.dma_start(t[:],
                                          xTd[dt * P:(dt + 1) * P,
                                              s0:s0 + SC])
                        st["xT"][dt] = t
                return f

            def q_load_wq(hb, dt4):
                def f():
                    wq = wqp.tile([P, 4, 2 * HEAD_DIM], BF16, tag="wq",
                                  name=f"wq{c}_{hb}_{dt4}")
                    eng = nc.scalar if (c == 0 and hb == 0 and dt4 == 0) \
                        else nc.sync
                    eng.dma_start(
                        wq[:], wqT[dt4 * 4 * P:(dt4 + 1) * 4 * P,
                                   hb * HEAD_DIM:(hb + 2) * HEAD_DIM
                                   ].rearrange("(d p) f -> p d f", p=P))
                    st["wq"][(hb, dt4)] = wq
                return f

            def q_load_wkv(which, dt4):
                def f():
                    w = wkvp.tile([P, 4, dkv], BF16, tag="wkv",
                                  name=f"w{which}{c}_{dt4}")
                    src = wkT if which == "k" else wvT
                    nc.sync.dma_start(
                        w[:], src[dt4 * 4 * P:(dt4 + 1) * 4 * P, :
                                  ].rearrange("(d p) f -> p d f", p=P))
                    st["w" + which][dt4] = w
                return f

            def q_mm_q(hb, dt4):
                def f():
                    if dt4 == 0:
                        pool, ptag = next_pool()
                        st[("pq", hb)] = [
                            pool.tile([P, SC], F32, tag=ptag,
                                      name=f"pq{c}_{hb}_{i}")
                            for i in range(2)]
                    pqs = st[("pq", hb)]
                    wq = st["wq"][(hb, dt4)]
                    for j in range(4):
                        dt = dt4 * 4 + j
                        for i in range(2):
                            nc.tensor.matmul(
                                pqs[i][:],
                                wq[:, j, i * HEAD_DIM:(i + 1) * HEAD_DIM],
                                st["xT"][dt][:],
                                start=(dt == 0), stop=(dt == ndt - 1))
                    if dt4 == ndt // 4 - 1:
                        for i in range(2):
                            rope_drain(qT[c][hb + i][:], pqs[i][:],
                                       st["cos"], st["sin"])
                        del st[("pq", hb)]
                return f

            def q_mm_kv(which, dt4):
                def f():
                    key = ("pk",) if which == "k" else ("pv",)
                    if dt4 == 0:
                        pool, ptag = next_pool()
                        st[key] = [
                            pool.tile([P, SC], F32, tag=ptag,
                                      name=f"p{which}{c}_{g}")
                            for g in range(hkv)]
                    ps = st[key]
                    w = st["w" + which][dt4]
                    for j in range(4):
                        dt = dt4 * 4 + j
                        for g in range(hkv):
                            nc.tensor.matmul(
                                ps[g][:],
                                w[:, j, g * HEAD_DIM:(g + 1) * HEAD_DIM],
                                st["xT"][dt][:],
                                start=(dt == 0), stop=(dt == ndt - 1))
                    if dt4 == ndt // 4 - 1:
                        if which == "k":
                            for g in range(hkv):
                                rope_drain(kT[c][g][:], ps[g][:],
                                           st["cos"], st["sin"])
                        else:
                            for g in range(hkv):
                                vt_sb = vsb.tile([P, SC], BF16, tag="vs")
                                nc.vector.tensor_copy(vt_sb[:], ps[g][:])
                                for t in range(SC // P):
                                    pt = ps_t.tile([P, P], BF16, tag="t")
                                    nc.tensor.transpose(
                                        pt[:], vt_sb[:, t * P:(t + 1) * P],
                                        iden_sb[:])
                                    nc.vector.tensor_copy(
                                        v_nat[g][c * (SC // P) + t][:], pt[:])
                        del st[key]
                return f

            # Build the quantum list. Loads appear in consumption order;
            # need[ci] = index of the last load that comps[ci] depends on.
            # Chunk 0 runs Q,K,V (attention starts with Q scores right
            # after); fill chunks run V,K,Q so the V-transpose chain and K
            # drains land early in the phase, never at the flush boundary.
            group_order = ["q", "k", "v"] if c == 0 else ["v", "k", "q"]
            loads = [q_load_x(0)]
            comps = []
            need = []

            def add(comp, *ls):
                loads.extend(ls)
                comps.append(comp)
                need.append(len(loads) - 1)

            for gi, grp in enumerate(group_order):
                if grp == "q":
                    for hb in range(0, hq, 2):
                        for dt4 in range(ndt // 4):
                            ls = [q_load_wq(hb, dt4)]
                            if gi == 0 and hb == 0 and dt4 == 0:
                                ls.append(q_cos)
                            if gi == 0 and hb == 0 and dt4 + 1 < ndt // 4:
                                ls.append(q_load_x(dt4 + 1))
                            add(q_mm_q(hb, dt4), *ls)
                else:
                    for dt4 in range(ndt // 4):
                        ls = [q_load_wkv(grp, dt4)]
                        if gi == 0 and dt4 == 0:
                            ls.append(q_cos)
                        if gi == 0 and dt4 + 1 < ndt // 4:
                            ls.append(q_load_x(dt4 + 1))
                        add(q_mm_kv(grp, dt4), *ls)
            quanta = []
            li = 0
            for ci, comp in enumerate(comps):
                # emit every load this comp needs, plus prefetch the loads
                # the NEXT comp needs (so DMA has a head start); no
                # prefetch before the very first comp (startup latency)
                tgt = need[ci] if ci == 0 else need[min(ci + 1, len(comps) - 1)]
                while li <= tgt:
                    quanta.append(loads[li])
                    li += 1
                quanta.append(comp)
            quanta.extend(loads[li:])
            return quanta

        def norm_quanta(cc, heads=range(hq), batched_recip=True):
            """Normalize ao[h][cc] in place: one reciprocal of the [8,512]
            denominator tile + per-head broadcast and multiply."""
            q = []
            heads = list(heads)
            h1 = heads[-1] + 1

            def recip():
                nc.vector.reciprocal(rc_c[cc][0:h1, :], dn_c[cc][0:h1, :])
            q.append(recip)
            for h in heads:
                def bm(h=h):
                    rrow = rowp.tile([1, SC], F32, tag="rrow", bufs=2)
                    nc.sync.dma_start(rrow[0:1, :], rc_c[cc][h:h + 1, :])
                    rb = rbp.tile([P, SC], F32, tag="rb")
                    nc.gpsimd.partition_broadcast(rb[:], rrow[0:1, :])
                    nc.vector.tensor_mul(ao[h][cc][:], ao[h][cc][:], rb[:])
                q.append(bm)
            return q

        def oproj_quanta(ccs):
            """O-projection quanta for the given chunks, m-tile major.

            wo loads run 2 m-tiles ahead of their matmuls, alternating the
            sync/scalar HWDGE queues; bf16 output halves store traffic."""
            st = {}

            def load(m):
                def f():
                    wo = wop.tile([P, hq, P], BF16, tag="wo",
                                  name=f"wo{m}_{ccs[0]}")
                    eng = nc.sync if m % 2 == 0 else nc.scalar
                    eng.dma_start(
                        wo[:], wot[m].rearrange("(o p) f -> p o f", p=P))
                    st[m] = wo
                return f

            def mm(m, cc):
                def f():
                    wo = st[m]
                    py = ps_p.tile([P, SC], F32, tag="p")
                    for o in range(hq):
                        nc.tensor.matmul(
                            py[:], wo[:, o, :], ao[o][cc][:],
                            start=(o == 0), stop=(o == hq - 1))
                    yo = vsb.tile([P, SC], BF16, tag="yo", bufs=4)
                    nc.vector.tensor_copy(yo[:], py[:])
                    eng = nc.scalar if m % 2 == 0 else nc.sync
                    eng.dma_start(
                        outT[m * P:(m + 1) * P, cc * SC:(cc + 1) * SC],
                        yo[:])
                return f

            nm = dim // P
            q = [load(0), load(1)]
            for m in range(nm):
                for cc in ccs:
                    q.append(mm(m, cc))
                if m + 2 < nm:
                    q.append(load(m + 2))
            return q

        def emit_attention(c, fill, reserve=0):
            s0 = c * SC
            nkt = (c + 1) * (SC // P)
            items = [(h, kt) for h in range(hq) for kt in range(nkt)]
            n_items = len(items)
            nf = len(fill)
            npaced = max(0, nf - reserve)
            fstate = [0]

            def drain_fill(i):
                tgt = npaced * (i + 1) // n_items
                while fstate[0] < tgt:
                    fill[fstate[0]]()
                    fstate[0] += 1

            pipe = {}

            def sc_ahead(i):
                h2, kt2 = items[i]
                g2 = h2 // nrep
                jlo = max(0, kt2 * P - s0)
                kc, ko = divmod(kt2, SC // P)
                pscr = ps_s.tile([P, SC], F32, tag="s",
                                 name=f"pscr{c}_{h2}_{kt2}")
                nc.tensor.matmul(
                    pscr[:, jlo:SC],
                    kT[kc][g2][:, ko * P:(ko + 1) * P],
                    qT[c][h2][:, jlo:SC],
                    start=True, stop=True)
                pipe[i] = pscr

            def head_end(h, acc, po):
                """Denominator reduce + drain for a finished head."""
                pd = ps_s.tile([P, SC], F32, tag="s", name=f"pd{c}_{h}")
                nc.tensor.matmul(pd[0:1, :], ones16_sb[:], acc[:],
                                 start=True, stop=True)
                dps = rowp.tile([1, SC], F32, tag="dps", bufs=3)
                nc.scalar.copy(dps[0:1, :], pd[0:1, :])
                nc.sync.dma_start(dn_c[c][h:h + 1, :], dps[0:1, :])
                nc.vector.tensor_copy(ao[h][c][:], po[:])
                return dps

            sc_ahead(0)
            if len(items) > 1:
                sc_ahead(1)
            accs = {}
            pos = {}
            last_dps = [None]
            for i, (h, kt) in enumerate(items):
                g = h // nrep
                if kt == 0:
                    accs[h] = accp.tile([P, SC], FP16, tag="acc",
                                        name=f"acc{c}_{h}")
                    pos[h] = ps_o.tile([P, SC], F32, tag="o",
                                       name=f"po{c}_{h}")
                acc, po = accs[h], pos[h]
                rr = kt * P - s0
                jlo = max(0, rr)
                if i + 2 < len(items):
                    sc_ahead(i + 2)
                pscr = pipe.pop(i)
                et = etp.tile([P, SC], FP16, tag="et",
                              name=f"et{c}_{h}_{kt}")
                nc.scalar.activation(
                    et[:, jlo:SC], pscr[:, jlo:SC],
                    mybir.ActivationFunctionType.Exp, scale=scale)
                if rr >= 0:
                    nc.vector.tensor_mul(et[:, jlo:jlo + P],
                                         et[:, jlo:jlo + P], tri_sb[:])
                if kt == 0:
                    nc.vector.tensor_copy(acc[:], et[:])
                else:
                    nc.vector.tensor_add(acc[:, jlo:SC],
                                         acc[:, jlo:SC], et[:, jlo:SC])
                nc.tensor.matmul(
                    po[:, jlo:SC],
                    v_nat[g][kt][:],
                    et[:, jlo:SC],
                    start=(kt == 0), stop=(kt == nkt - 1))
                if kt == nkt - 1 and h < hq - 1:
                    # fill first: gives ACT exp + DVE add time to finish
                    # before the PE's ones-matmul consumes acc
                    drain_fill(i)
                    head_end(h, acc, po)
                    del accs[h], pos[h]
                    if c == nch - 1 and h == hq - 2:
                        # heads 0..hq-2 done: normalize them while the
                        # last head's items run
                        for f in norm_quanta(c, range(hq - 1)):
                            f()
                elif kt == nkt - 1:
                    # final head: flush most of the reserved fill first so
                    # the denominator chain overlaps real PE work, but hold
                    # back a few quanta to also cover the post-head_end
                    # normalize chain
                    hold = 12 if c == nch - 1 else 0
                    while fstate[0] < nf - hold:
                        fill[fstate[0]]()
                        fstate[0] += 1
                    if c == nch - 1:
                        last_dps[0] = head_end(h, acc, po)
                        while fstate[0] < nf:
                            fill[fstate[0]]()
                            fstate[0] += 1
                    else:
                        # defer into the next chunk's fill: the pd matmul
                        # then sits behind the next chunk's first scores
                        def deferred(h=h, acc=acc, po=po):
                            head_end(h, acc, po)
                        last_dps[0] = deferred
                    del accs[h], pos[h]
                else:
                    drain_fill(i)
            while fstate[0] < nf:
                fill[fstate[0]]()
                fstate[0] += 1
            return last_dps[0]

        # ---- schedule ----
        for f in proj_quanta(0):
            f()
        dps_last = None
        for c in range(nch):
            if c < nch - 1:
                fill = list(proj_quanta(c + 1))
                if c >= 1:
                    fill = norm_quanta(c - 1) + fill
            else:
                fill = norm_quanta(c - 1) + oproj_quanta([0, 1, 2])
            if dps_last is not None:
                fill = [dps_last] + fill
            dps_last = emit_attention(c, fill, reserve=max(8, len(fill) // 8))
        # tail: last head of last chunk (reciprocal straight off the
        # partition-0 scratch row, no SBUF round-trip) + O-proj of chunk 3
        rlast = rowp.tile([1, SC], F32, tag="rrow", bufs=2)
        nc.vector.reciprocal(rlast[0:1, :], dps_last[0:1, :])
        rb = rbp.tile([P, SC], F32, tag="rb")
        nc.gpsimd.partition_broadcast(rb[:], rlast[0:1, :])
        nc.vector.tensor_mul(ao[hq - 1][nch - 1][:],
                             ao[hq - 1][nch - 1][:], rb[:])
        for f in oproj_quanta([nch - 1]):
            f()

    nc.compile()
    return nc


def make_core_inputs(data, Wq, Wk, Wv, Wo, cos, sin):
    """Build in_maps for the 8 cores. Core id = 4*b + g."""
    bf = ml_dtypes.bfloat16

    def cbf(a):
        return np.ascontiguousarray(np.asarray(a).astype(bf))

    c = np.ascontiguousarray
    dq = HQ * HEAD_DIM
    dkv = HKV * HEAD_DIM
    dim = Wq.shape[1]
    tri_m = np.triu(np.ones((P, P), dtype=np.float16))
    iden = np.eye(P, dtype=bf)
    ones_col = np.ones((P, 1), dtype=np.float32)
    ones16 = np.ones((P, 1), dtype=np.float16)
    cosT = c(cos.T.astype(np.float32))
    sinT = c(sin.T.astype(np.float32))
    xt_by_batch = [cbf(data[b].T) for b in range(data.shape[0])]
    in_maps = []
    for core in range(N_CORES):
        b, g = divmod(core, 4)
        qs = slice(g * dq, (g + 1) * dq)
        ks = slice(g * dkv, (g + 1) * dkv)
        woT = Wo[:, qs].T                        # [dq, dim]
        wot = cbf(woT.reshape(dq, dim // P, P).transpose(1, 0, 2))
        in_maps.append({
            "xT": xt_by_batch[b],
            "wqT": cbf(Wq[qs, :].T),
            "wkT": cbf(Wk[ks, :].T),
            "wvT": cbf(Wv[ks, :].T),
            "wot": wot,
            "cosT": cosT,
            "sinT": sinT,
            "tri": tri_m,
            "iden": iden,
            "ones_col": ones_col,
            "ones16": ones16,
        })
    return in_maps


_COMPILED = {}


def _get_program():
    key = (SEQ, DIM, HQ, HKV)
    if key not in _COMPILED:
        _COMPILED[key] = build_program()
    return _COMPILED[key]


def run(inputs, trace=False, tmpdir=None, trace_cores=None):
    nc = _get_program()
    in_maps = make_core_inputs(
        inputs["data"], inputs["Wq"], inputs["Wk"], inputs["Wv"],
        inputs["Wo"], inputs["cos"], inputs["sin"])
    kw = {}
    if trace:
        kw = dict(trace=True, tmpdir=tmpdir, trace_cores=trace_cores)
    res = run_bass_kernel_spmd(nc, in_maps, list(range(N_CORES)), **kw)
    B = inputs["data"].shape[0]
    out = np.zeros((B, SEQ, DIM), dtype=np.float32)
    for core in range(N_CORES):
        b = core // 4
        out[b] += np.asarray(res.results[core]["outT"]).astype(np.float32).T
    return out, res


def kernel(data, Wq, Wk, Wv, Wo, cos, sin, mask):
    assert np.asarray(mask).size == 1, "only causal (numel==1) mask supported"
    inputs = {
        "data": np.asarray(data, dtype=np.float32),
        "Wq": np.asarray(Wq, dtype=np.float32),
        "Wk": np.asarray(Wk, dtype=np.float32),
        "Wv": np.asarray(Wv, dtype=np.float32),
        "Wo": np.asarray(Wo, dtype=np.float32),
        "cos": np.asarray(cos, dtype=np.float32),
        "sin": np.asarray(sin, dtype=np.float32),
    }
    out, _ = run(inputs)
    return out
